# revision 1
# baseline (speedup 1.0000x reference)
"""AttnBlock (GroupNorm + cross-attention + proj + residual) on 8 trn2 cores.

Problem (hardcoded shapes): x, c: [2, 128, 16, 16, 16] fp32; C=128 channels,
N=4096 spatial tokens, 4 groups of 32 channels.

  h  = GN(x; g1, b1)            c_ = GN(c; g2, b2)
  q = wq c_ + bq ; k = wk h + bk ; v = wv h + bv
  S[b,i,j] = <q[:,i], k[:,j]> / sqrt(C) ;  A = softmax_j(S)
  out = x + wp (v A^T) + bp

Sharding: 8 cores, core m -> batch b=m//4, query rows i0=(m%4)*1024 .. +1024.
Each core recomputes GN + K/V^T for its batch (cheap), computes its
[1024 x 4096] slice of exp(S^T) with j on partitions (transpose-free layout),
accumulates V^T @ P and the softmax denominator in PSUM, normalizes, projects,
adds residual, and returns its [128, 1024] output slice.

Matmul operands are bf16 (fp32 runs at 1/4 PE rate; bf16 weight loads
pipeline). Group-norm statistics and the softmax denominator/reciprocal
path stay fp32/f32r. Softmax denominators accumulate via 4 concurrently
executing col-tiled ones-matmuls (tile_position); 1/d = exp(-ln(d)) on
ScalarE; per-partition broadcast via K=1 matmul.
"""

import ml_dtypes
import numpy as np

import concourse.bass as bass
import concourse.tile as tile
from concourse import mybir
from concourse.bass_utils import run_bass_kernel_spmd

def _patch_walrus_flags():
    """Re-enable walrus's LDWEIGHTS optimization (hardcoded off in
    bass_utils); without it every matmul serializes a full weight load."""
    import concourse.bass_utils as bu
    if getattr(bu, "_ldw_patched", False):
        return
    bu._ldw_patched = True  # ldw-opt rejects bass-emitted InstLdweights; keep off


_patch_walrus_flags()

N_CORES = 8
C = 128
N = 4096          # tokens per batch
I = 1024          # query rows per core
NG = 4            # groups
EPS = 1e-6
SCALE = 1.0 / np.sqrt(C)
JB = N // 128     # 32 j-blocks
F32 = mybir.dt.float32
F32R = mybir.dt.float32r

BF16 = mybir.dt.bfloat16

MM_DT = BF16      # projections matmul operand dtype
ATTN_DT = BF16    # scores / AV / denom matmul operand dtype

DEBUG_OUTS = False


class SlimTC(tile.TileContext):
    """TileContext with a slimmer kernel-tail: one all-engine barrier instead
    of two.  The second barrier only orders other engines behind the sem
    clears; each engine's own stream still completes before halt, and NRT
    doesn't restart streams until all engines halt, so reruns stay safe."""

    def _drain_and_barrier(self, tick_clock, wait_clock):
        from concourse.vector_clock import ScopedClock
        drain_inst = self.nc.sync.drain()
        wait_clock.add_sem_waits(
            drain_inst.ins, ScopedClock({None: tick_clock.global_clock})
        )
        # the drain's waits cover every proc's final tick; hand off to
        # gpsimd (which owns the sem clears) with one semaphore instead of
        # the full EVSEM butterfly barrier
        done = self.nc.alloc_semaphore("tail_done")
        drain_inst.then_inc(done)
        self.nc.gpsimd.wait_ge(done, 1)
        assert self.sems is not None
        popped = self.nc._tile_sem_poison_stack.pop()
        assert popped is self._sem_poison
        sems = list(self.sems.allocated().values()) + [done]
        self.nc.clear_and_free_semaphores(sems)


def cap_sync_waits(nc):
    """Split multi-wait instructions: the pinned walrus accepts at most one
    sync wait per instruction ("Too many sync wait commands"). Hoist extra
    waits into single-wait NOPs inserted just before, on the same engine."""
    ctr = 0
    for f in nc.m.functions:
        for b in f.blocks:
            out = []
            for inst in b.instructions:
                si = inst.sync_info
                if si is not None and si.on_wait and len(si.on_wait) > 1:
                    waits = list(si.on_wait)
                    for w in waits[:-1]:
                        ctr += 1
                        out.append(mybir.InstNoOp(
                            name=f"I-waitsplit-{ctr}",
                            engine=inst.engine,
                            bass_nofuse=True,
                            sync_info=mybir.SyncInfo(on_wait=[w], on_update=[]),
                        ))
                    si.on_wait = waits[-1:]
                out.append(inst)
            b.instructions = out


def _r(ap):
    """View an fp32 AP as float32r for full-rate PE matmuls."""
    if MM_DT is F32:
        return ap
    return ap.bitcast(MM_DT)


def build_program():
    nc = bass.Bass("TRN2", target_bir_lowering=False, debug=False)

    # I/O.  xb/cb arrive ROTATED per core (columns rolled by -i0) so the
    # query/residual slice is always columns 0:1024; attention is
    # permutation-invariant in j, so k/v/P computed in the rotated frame
    # give the same output for these query rows.
    xb = nc.declare_dram_parameter("xb", [C, N], F32, isOutput=False)
    cb = nc.declare_dram_parameter("cb", [C, N], BF16, isOutput=False)
    # packed weights [C, 4C]: wqT | wkT | wvT | wpT
    wall = nc.declare_dram_parameter("wall", [C, 4 * C], F32, isOutput=False)
    # packed per-channel vectors [C, 14]:
    # 0:bq 1:bk 2:btp 3:g1 4:b1 5:g2 6:b2 7:ones 8-11:gavg 12:sel_e 13:sel_o
    vall = nc.declare_dram_parameter("vall", [C, 14], F32, isOutput=False)
    # bc4 group-broadcast indicator rows
    rall = nc.declare_dram_parameter("rall", [NG, C], F32, isOutput=False)
    y = nc.declare_dram_parameter("y", [C, I], F32, isOutput=True)

    NCH = 8            # 512-wide chunks per [C, N] tensor
    CH = N // NCH

    with SlimTC(nc) as tc:
        with (
            tc.tile_pool(name="persist", bufs=1) as per,
            tc.tile_pool(name="smalls", bufs=1) as sm,
            tc.tile_pool(name="ptiles", bufs=6) as pp,
        ):
            eps128_t = sm.tile([C, 1], F32, tag="eps128")
            nc.vector.memset(eps128_t[:], EPS)
            zero128_t = sm.tile([C, 1], F32, tag="zero128")
            nc.vector.memset(zero128_t[:], 0.0)
            zero1_t = sm.tile([1, 1], F32, tag="zero1")
            nc.vector.memset(zero1_t[:], 0.0)
            # warm the ACT table set (Ln+Exp) off the critical path
            warm_t = sm.tile([1, 1], F32, tag="warm")
            nc.vector.memset(warm_t[:], 1.0)
            nc.scalar.activation(out=warm_t[:], in_=warm_t[:],
                                 func=mybir.ActivationFunctionType.Ln,
                                 bias=zero1_t[:], scale=1.0)
            nc.scalar.activation(out=warm_t[:], in_=warm_t[:],
                                 func=mybir.ActivationFunctionType.Exp,
                                 bias=zero1_t[:], scale=1.0)

            # ---- chunked input DMA + per-chunk bn_stats (overlap) ----
            x_t = per.tile([C, N], F32, tag="x")
            c_t = per.tile([C, N], BF16, tag="c")
            stats_c = sm.tile([C, NCH, 6], F32, tag="stats_c")
            # c on the sync HW-DGE queue (DVE bn_stats), x on the gpsimd
            # queue in parallel.  Two contiguous half-tensor transfers per
            # input (16KB/partition rows DMA much faster than 2KB chunks);
            # x stats as Identity/Square+accum on the otherwise-idle ScalarE.
            scr_t = per.tile([C, I], F32, tag="scr")
            sxs = sm.tile([C, 2, 4], F32, tag="sxs")
            for hf in range(2):
                hsl = slice(hf * (N // 2), (hf + 1) * (N // 2))
                nc.sync.dma_start(c_t[:, hsl], cb[:, hsl])
                nc.gpsimd.dma_start(x_t[:, hsl], xb[:, hsl])
                for ch4 in range(4):
                    ch = hf * 4 + ch4
                    nc.vector.bn_stats(
                        out=stats_c[:, ch, :],
                        in_=c_t[:, ch * CH:(ch + 1) * CH],
                    )
                for qr4 in range(2):
                    qr = hf * 2 + qr4
                    qsl = slice(qr * I, (qr + 1) * I)
                    # sum(x) on DVE, sum(x^2) on ScalarE -- parallel engines
                    nc.vector.tensor_reduce(
                        out=sxs[:, 0, qr:qr + 1], in_=x_t[:, qsl],
                        axis=mybir.AxisListType.X, op=mybir.AluOpType.add,
                    )
                    nc.scalar.activation(
                        out=scr_t[:], in_=x_t[:, qsl],
                        func=mybir.ActivationFunctionType.Square,
                        bias=zero128_t[:], scale=1.0,
                        accum_out=sxs[:, 1, qr:qr + 1],
                    )

            # ---- packed constant loads (3 DMAs, ahead of x/c in the queue) ----
            wall_t = per.tile([C, 4 * C], F32, tag="wall")
            nc.sync.dma_start(wall_t[:], wall[:])
            vall_t = sm.tile([C, 14], F32, tag="vall")
            nc.sync.dma_start(vall_t[:], vall[:])
            rall_t = sm.tile([NG, C], F32, tag="rall")
            nc.sync.dma_start(rall_t[:], rall[:])

            wq_t = wall_t[:, 0 * C:1 * C]
            wk_t = wall_t[:, 1 * C:2 * C]
            wv_t = wall_t[:, 2 * C:3 * C]
            wp_t = wall_t[:, 3 * C:4 * C]
            bq_t = vall_t[:, 0:1]
            bk_t = vall_t[:, 1:2]
            btp_t = vall_t[:, 2:3]
            g1_t = vall_t[:, 3:4]
            b1_t = vall_t[:, 4:5]
            g2_t = vall_t[:, 5:6]
            b2_t = vall_t[:, 6:7]
            ones_t = vall_t[:, 7:8]
            gavg_t = vall_t[:, 8:12]
            bc4_t = rall_t[:]
            ones1_t = sm.tile([1, C], F32, tag="ones1")
            nc.vector.memset(ones1_t[:], 1.0)

            # rounded copies for the PE
            wq_r = per.tile([C, C], MM_DT, tag="wq_r")
            nc.vector.tensor_copy(wq_r[:], wq_t[:])
            wk_r = per.tile([C, C], MM_DT, tag="wk_r")
            nc.vector.tensor_copy(wk_r[:], wk_t[:])
            wv_r = per.tile([C, C], MM_DT, tag="wv_r")
            nc.vector.tensor_copy(wv_r[:], wv_t[:])
            wp_r = per.tile([C, C], MM_DT, tag="wp_r")
            nc.vector.tensor_copy(wp_r[:], wp_t[:])
            ones_a = sm.tile([C, 1], ATTN_DT, tag="ones_a")
            nc.vector.tensor_copy(ones_a[:], ones_t[:])
            ones_r = sm.tile([C, 1], F32R, tag="ones_r")
            nc.vector.tensor_copy(ones_r[:], ones_t[:])
            ones1_r = sm.tile([1, C], F32R, tag="ones1_r")
            nc.vector.tensor_copy(ones1_r[:], ones1_t[:])
            sele_r = sm.tile([C, 1], F32R, tag="sele_r")
            nc.vector.tensor_copy(sele_r[:], vall_t[:, 12:13])
            selo_r = sm.tile([C, 1], F32R, tag="selo_r")
            nc.vector.tensor_copy(selo_r[:], vall_t[:, 13:14])

            # ---- group-norm channel affine A[c], B[c] ----
            # d2 = [mean_c, E[x^2]_c] per channel; group aggregation via tiny
            # indicator matmuls; rstd as exp(-0.5*ln(var+eps))
            def gn_affine_d2(d2, gamma_t, beta_t, label):
                with tc.tile_pool(
                    name=f"gnps_{label}", bufs=1, space=bass.MemorySpace.PSUM
                ) as gnps:
                    gps = gnps.tile([NG, 2], F32, tag="g")
                    nc.tensor.matmul(gps[:], gavg_t[:], d2[:], start=True, stop=True)
                    gsb = sm.tile([NG, 2], F32, tag=f"gsb_{label}")
                    nc.vector.tensor_copy(gsb[:], gps[:])
                    cps = gnps.tile([C, 2], F32, tag="ch")
                    nc.tensor.matmul(cps[:], bc4_t[:], gsb[:], start=True, stop=True)
                    csb = sm.tile([C, 2], F32, tag=f"csb_{label}")
                    nc.vector.tensor_copy(csb[:], cps[:])
                var = sm.tile([C, 1], F32, tag=f"var_{label}")
                nc.vector.tensor_mul(var[:], csb[:, 0:1], csb[:, 0:1])
                nc.vector.tensor_sub(var[:], csb[:, 1:2], var[:])
                lnv = sm.tile([C, 1], F32, tag=f"lnv_{label}")
                nc.scalar.activation(
                    out=lnv[:], in_=var[:], func=mybir.ActivationFunctionType.Ln,
                    bias=eps128_t[:], scale=1.0,
                )
                rstd = sm.tile([C, 1], F32, tag=f"rstd_{label}")
                nc.scalar.activation(
                    out=rstd[:], in_=lnv[:], func=mybir.ActivationFunctionType.Exp,
                    bias=zero128_t[:], scale=-0.5,
                )
                a_t = sm.tile([C, 1], F32, tag=f"A_{label}")
                nc.vector.tensor_mul(a_t[:], rstd[:], gamma_t[:])
                b_t = sm.tile([C, 1], F32, tag=f"B_{label}")
                nc.vector.tensor_mul(b_t[:], csb[:, 0:1], a_t[:])
                nc.vector.tensor_sub(b_t[:], beta_t[:], b_t[:])
                return a_t, b_t

            # c path: bn_aggr -> [mean, mean^2+var]
            mv = sm.tile([C, 2], F32, tag="mv_c")
            nc.vector.bn_aggr(out=mv[:], in_=stats_c[:])
            d2c = sm.tile([C, 2], F32, tag="d2_c")
            nc.vector.tensor_copy(d2c[:, 0:1], mv[:, 0:1])
            nc.vector.tensor_mul(d2c[:, 1:2], mv[:, 0:1], mv[:, 0:1])
            nc.vector.tensor_add(d2c[:, 1:2], d2c[:, 1:2], mv[:, 1:2])
            ac_t, bc_t = gn_affine_d2(d2c, g2_t, b2_t, "c")

            # x path: chunk sums -> means
            d2x = sm.tile([C, 2], F32, tag="d2_x")
            nc.vector.tensor_reduce(
                out=d2x[:], in_=sxs[:],
                axis=mybir.AxisListType.X, op=mybir.AluOpType.add,
            )
            nc.vector.tensor_scalar_mul(d2x[:], d2x[:], 1.0 / float(N))
            ax_t, bx_t = gn_affine_d2(d2x, g1_t, b1_t, "x")

            # ---- chunked normalize + projections ----
            # h chunk -> k chunk (matmul) and vT blocks (h stationary)
            h_t = per.tile([C, N], MM_DT, tag="h")
            k_t = per.tile([C, N], ATTN_DT, tag="k")
            q_t = per.tile([C, I], ATTN_DT, tag="q")
            vt_t = per.tile([C, JB, C], ATTN_DT, tag="vt")

            with tc.tile_pool(
                name="proj_ps", bufs=2, space=bass.MemorySpace.PSUM
            ) as pps:
                # cn / q for the first 1024 (rotated) columns of c
                cn_t = per.tile([C, I], MM_DT, tag="cn")
                nc.vector.tensor_scalar(
                    out=cn_t[:], in0=c_t[:, 0:I], scalar1=ac_t[:], scalar2=bc_t[:],
                    op0=mybir.AluOpType.mult, op1=mybir.AluOpType.add,
                )
                qps = pps.tile([C, I], F32, tag="q")
                for ih in range(2):
                    nc.tensor.matmul(
                        qps[:, ih * 512:(ih + 1) * 512],
                        wq_r[:], cn_t[:, ih * 512:(ih + 1) * 512],
                        start=True, stop=True,
                    )
                nc.scalar.activation(
                    out=q_t[:], in_=qps[:],
                    func=mybir.ActivationFunctionType.Identity,
                    bias=bq_t[:], scale=1.0,
                )
                for ch in range(NCH):
                    sl = slice(ch * CH, (ch + 1) * CH)
                    if ch % 2 == 0:
                        wsl = slice(ch * CH, (ch + 2) * CH)
                        nc.vector.tensor_scalar(
                            out=h_t[:, wsl], in0=x_t[:, wsl], scalar1=ax_t[:],
                            scalar2=bx_t[:],
                            op0=mybir.AluOpType.mult, op1=mybir.AluOpType.add,
                        )
                    kps = pps.tile([C, CH], F32, tag="kq")
                    nc.tensor.matmul(kps[:], wk_r[:], h_t[:, sl],
                                     start=True, stop=True)
                    # bias-add on ScalarE (keeps DVE free for stats/normalize)
                    nc.scalar.activation(
                        out=k_t[:, sl], in_=kps[:],
                        func=mybir.ActivationFunctionType.Identity,
                        bias=bk_t[:], scale=1.0,
                    )
                    # 4 vT blocks share one psum bank -> one wide copy
                    vps = pps.tile([C, 4, C], F32, tag="vt")
                    for j4 in range(4):
                        jb = ch * 4 + j4
                        nc.tensor.matmul(
                            vps[:, j4, :], h_t[:, jb * 128:(jb + 1) * 128],
                            wv_r[:], start=True, stop=True,
                        )
                    nc.vector.tensor_copy(
                        vt_t[:, ch * 4:(ch + 1) * 4, :], vps[:]
                    )

            # ---- attention ----
            # Software-pipelined: scores for jb+2 are emitted ahead of the
            # exp-dependent AV/denom work for jb, so the PE never stalls on
            # the ScalarE exp.  Denominators accumulate into 4 separate
            # col-group accumulators (tile_position) so 4 ones-matmuls run
            # concurrently on the PE array.
            o_sb = per.tile([C, I], MM_DT, tag="osb")
            rb_sb = per.tile([C, I], F32, tag="rbsb")
            f_t = per.tile([C, I], F32, tag="f")
            zz_t = per.tile([C, I], F32, tag="zz")
            d4s = per.tile([C, 512], F32R, tag="d4s")

            st_tiles = {}
            p_tiles = {}

            with tc.tile_pool(
                name="acc_ps", bufs=1, space=bass.MemorySpace.PSUM
            ) as acc:
                o_ps = acc.tile([C, I], F32, tag="o")
                d4_ps = acc.tile([C, 512], F32, tag="d4")
                nc.vector.memset(d4_ps[:], 0.0)

                with tc.tile_pool(
                    name="st_ps", bufs=2, space=bass.MemorySpace.PSUM
                ) as stp:
                    def emit_scores(jb):
                        st = stp.tile([C, I], F32, tag="st")
                        st_tiles[jb] = st
                        for ih in range(2):
                            nc.tensor.matmul(
                                st[:, ih * 512:(ih + 1) * 512],
                                k_t[:, jb * 128:(jb + 1) * 128],
                                q_t[:, ih * 512:(ih + 1) * 512],
                                start=True, stop=True,
                            )

                    def emit_exp(jb):
                        p_t = pp.tile([C, I], ATTN_DT, tag="p")
                        p_tiles[jb] = p_t
                        nc.scalar.activation(
                            out=p_t[:], in_=st_tiles.pop(jb)[:],
                            func=mybir.ActivationFunctionType.Exp,
                            bias=zero128_t[:], scale=float(SCALE),
                        )

                    emit_scores(0)
                    emit_scores(1)
                    emit_exp(0)
                    for jb in range(JB):
                        if jb + 2 < JB:
                            emit_scores(jb + 2)
                        if jb + 1 < JB:
                            emit_exp(jb + 1)
                        p_t = p_tiles[jb]
                        first, last = jb == 0, jb == JB - 1
                        for ih in range(2):
                            sl = slice(ih * 512, (ih + 1) * 512)
                            nc.tensor.matmul(
                                o_ps[:, sl], vt_t[:, jb, :], p_t[:, sl],
                                start=first, stop=last,
                            )
                        if jb % 2 == 1:
                            for g in range(4):
                                jj, ih = jb - 1 + g // 2, g % 2
                                sl = slice(ih * 512, (ih + 1) * 512)
                                nc.tensor.matmul(
                                    d4_ps[32 * g:32 * g + 1, 0:512],
                                    ones_a[:], p_tiles[jj][:, sl],
                                    start=jb == 1, stop=last,
                                    tile_position=(0, 32 * g),
                                )
                            p_tiles.pop(jb - 1)
                            p_tiles.pop(jb)

                # O out of PSUM + projection immediately (PE/DVE work in
                # parallel with the reciprocal chain below)
                # evacuate O on the post-exp-idle ScalarE so DVE can run
                # the denominator collapse in parallel
                nc.scalar.activation(
                    out=o_sb[:, 0:512], in_=o_ps[:, 0:512],
                    func=mybir.ActivationFunctionType.Identity,
                    bias=zero128_t[:], scale=1.0,
                )
                nc.scalar.activation(
                    out=o_sb[:, 512:1024], in_=o_ps[:, 512:1024],
                    func=mybir.ActivationFunctionType.Identity,
                    bias=zero128_t[:], scale=1.0,
                )

                # one copy moves all 4 denominator rows (other partitions
                # are memset zeros)
                nc.vector.tensor_copy(d4s[:], d4_ps[:])

            with tc.tile_pool(
                name="tail_ps", bufs=1, space=bass.MemorySpace.PSUM
            ) as tlp:
                z_ps = tlp.tile([C, I], F32, tag="z")
                for ih in range(2):
                    sl = slice(ih * 512, (ih + 1) * 512)
                    nc.tensor.matmul(z_ps[:, sl], wp_r[:], o_sb[:, sl],
                                     start=True, stop=True)

                d_fin = tlp.tile([1, I], F32, tag="dfin")
                # rows {0,64} hold i 0:512, rows {32,96} hold 512:1024
                nc.tensor.matmul(
                    d_fin[:, 0:512], sele_r[:], d4s[:], start=True, stop=True,
                )
                nc.tensor.matmul(
                    d_fin[:, 512:1024], selo_r[:], d4s[:], start=True, stop=True,
                )
                lnd = sm.tile([1, I], F32, tag="lnd")
                nc.scalar.activation(
                    out=lnd[:], in_=d_fin[:],
                    func=mybir.ActivationFunctionType.Ln, bias=zero1_t[:],
                    scale=1.0,
                )
                rsb = sm.tile([1, I], F32R, tag="rsb")
                nc.scalar.activation(
                    out=rsb[:], in_=lnd[:],
                    func=mybir.ActivationFunctionType.Exp, bias=zero1_t[:],
                    scale=-1.0,
                )
                rb_ps = tlp.tile([C, I], F32, tag="rb")
                for ih in range(2):
                    sl = slice(ih * 512, (ih + 1) * 512)
                    nc.tensor.matmul(
                        rb_ps[:, sl], ones1_r[:], rsb[:, sl],
                        start=True, stop=True,
                    )
                    nc.vector.tensor_copy(rb_sb[:, sl], rb_ps[:, sl])
                    # f = (z * recip + btp) + x  in two DVE ops
                    nc.vector.tensor_tensor(
                        zz_t[:, sl], z_ps[:, sl], rb_sb[:, sl],
                        mybir.AluOpType.mult,
                    )
                    nc.vector.scalar_tensor_tensor(
                        out=f_t[:, sl], in0=zz_t[:, sl], scalar=btp_t[:],
                        in1=x_t[:, sl],
                        op0=mybir.AluOpType.add, op1=mybir.AluOpType.add,
                    )
                    nc.sync.dma_start(y[:, sl], f_t[:, sl])

    cap_sync_waits(nc)
    return nc


_PROGRAM = None


def _get_program():
    global _PROGRAM
    if _PROGRAM is None:
        _PROGRAM = build_program()
    return _PROGRAM


def _prep_in_maps(x, c, g1, b1, g2, b2, wq, bq, wk, bk, wv, bv, wp, bp):
    f = np.float32
    a = lambda v: np.asarray(v, f)
    ch = np.arange(C) // 32
    gavg = np.zeros((C, NG), f)
    gavg[np.arange(C), ch] = 1.0 / 32.0
    bc4 = np.zeros((NG, C), f)
    bc4[ch, np.arange(C)] = 1.0
    wall = np.concatenate([a(wq).T, a(wk).T, a(wv).T, a(wp).T], axis=1)
    vall = np.stack([
        a(bq), a(bk), a(wp) @ a(bv) + a(bp), a(g1), a(b1), a(g2), a(b2),
        np.ones(C, f),
    ], axis=1)
    sel_e = np.zeros((C, 1), f); sel_e[0] = 1.0; sel_e[64] = 1.0
    sel_o = np.zeros((C, 1), f); sel_o[32] = 1.0; sel_o[96] = 1.0
    vall = np.concatenate([vall, gavg, sel_e, sel_o], axis=1)   # [C, 14]
    rall = bc4
    common = {
        "wall": np.ascontiguousarray(wall),
        "vall": np.ascontiguousarray(vall),
        "rall": np.ascontiguousarray(rall),
    }
    xf = a(x).reshape(2, C, N)
    cf = a(c).reshape(2, C, N)
    in_maps = []
    for m in range(N_CORES):
        b, quarter = m // 4, m % 4
        i0 = quarter * I
        # roll columns so this core's query/residual rows are columns 0:I;
        # attention is permutation-invariant in j so the rotated frame is safe
        in_maps.append({
            "xb": np.ascontiguousarray(np.roll(xf[b], -i0, axis=1)),
            "cb": np.ascontiguousarray(
                np.roll(cf[b], -i0, axis=1)).astype(ml_dtypes.bfloat16),
            **common,
        })
    return in_maps


def run_spmd(inputs, trace=False, **kw):
    nc = _get_program()
    in_maps = _prep_in_maps(**inputs)
    return run_bass_kernel_spmd(nc, in_maps, list(range(N_CORES)), trace=trace, **kw)


def kernel(**inputs) -> np.ndarray:
    res = run_spmd(inputs, trace=False)
    out = np.empty((2, C, N), np.float32)
    for m in range(N_CORES):
        b, quarter = m // 4, m % 4
        out[b][:, quarter * I:(quarter + 1) * I] = res.results[m]["y"]
    return out.reshape(2, C, 16, 16, 16)



# revision 12
# speedup vs baseline: 1.0544x; 1.0544x over previous
"""AttnBlock (GroupNorm + cross-attention + proj + residual) on 8 trn2 cores.

Problem (hardcoded shapes): x, c: [2, 128, 16, 16, 16] fp32; C=128 channels,
N=4096 spatial tokens, 4 groups of 32 channels.

  h  = GN(x; g1, b1)            c_ = GN(c; g2, b2)
  q = wq c_ + bq ; k = wk h + bk ; v = wv h + bv
  S[b,i,j] = <q[:,i], k[:,j]> / sqrt(C) ;  A = softmax_j(S)
  out = x + wp (v A^T) + bp

Folded formulation (x-side GroupNorm folded through the attention algebra,
so attention runs directly on RAW bf16 x):
  With D = diag(A1), hn = A1*x + B1 (per-channel affine from GN stats):
    scores  S'[j,i] = x[:,j]^T q'[:,i],  q' = D Wk^T q  (the per-i constant
            beta_k^T q drops out of softmax_j)
    q = Wq cn + bq  =>  q' = D (Wk^T Wq cn + Wk^T bq) = D (G cn + g0)
    U = X P  (raw x as values);  out = x + M D (U/d) + beta_f,
            M = Wp Wv,  beta_f = M B1 + Wp bv + bp
  Host packs G^T = Wq^T Wk and M^T; everything else is computed on chip.

Sharding: 8 cores, core m -> batch b=m//4, query rows i0=(m%4)*1024 .. +1024.
Inputs arrive column-rotated so the core's query/residual slice is cols 0:I.
Per core: DMA raw bf16 x/c, GN stats + x-block transposes overlap the DMA,
then a software-pipelined scores->exp->AV loop (exp on ACT is the wall),
then a short tail (U scale, M matmul, DVE reciprocal, combine, DMA out).
"""

import ml_dtypes
import numpy as np

import concourse.bass as bass
import concourse.tile as tile
from concourse import mybir
from concourse.bass_utils import run_bass_kernel_spmd

N_CORES = 8
C = 128
N = 4096          # tokens per batch
I = 1024          # query rows per core
NG = 4            # groups
EPS = 1e-6
SCALE = 1.0 / np.sqrt(C)
JB = N // 128     # 32 j-blocks
F32 = mybir.dt.float32
F32R = mybir.dt.float32r
BF16 = mybir.dt.bfloat16

MM_DT = BF16
ATTN_DT = BF16

N_WARM = 4        # PE p-state warm matmuls between projections and scores

DEBUG_OUTS = False


class SlimTC(tile.TileContext):
    """TileContext with a slimmer kernel-tail: one all-engine barrier instead
    of two.  The second barrier only orders other engines behind the sem
    clears; each engine's own stream still completes before halt, and NRT
    doesn't restart streams until all engines halt, so reruns stay safe."""

    def _drain_and_barrier(self, tick_clock, wait_clock):
        from concourse.vector_clock import ScopedClock
        drain_inst = self.nc.sync.drain()
        wait_clock.add_sem_waits(
            drain_inst.ins, ScopedClock({None: tick_clock.global_clock})
        )
        done = self.nc.alloc_semaphore("tail_done")
        drain_inst.then_inc(done)
        self.nc.gpsimd.wait_ge(done, 1)
        assert self.sems is not None
        popped = self.nc._tile_sem_poison_stack.pop()
        assert popped is self._sem_poison
        sems = list(self.sems.allocated().values()) + [done]
        self.nc.clear_and_free_semaphores(sems)


def cap_sync_waits(nc):
    """Split multi-wait instructions: the pinned walrus accepts at most one
    sync wait per instruction ("Too many sync wait commands"). Hoist extra
    waits into single-wait NOPs inserted just before, on the same engine."""
    ctr = 0
    for f in nc.m.functions:
        for b in f.blocks:
            out = []
            for inst in b.instructions:
                si = inst.sync_info
                if si is not None and si.on_wait and len(si.on_wait) > 1:
                    waits = list(si.on_wait)
                    for w in waits[:-1]:
                        ctr += 1
                        out.append(mybir.InstNoOp(
                            name=f"I-waitsplit-{ctr}",
                            engine=inst.engine,
                            bass_nofuse=True,
                            sync_info=mybir.SyncInfo(on_wait=[w], on_update=[]),
                        ))
                    si.on_wait = waits[-1:]
                out.append(inst)
            b.instructions = out


def build_program():
    nc = bass.Bass("TRN2", target_bir_lowering=False, debug=False)

    # I/O.  xb/cb arrive ROTATED per core (columns rolled by -i0) so the
    # query/residual slice is always columns 0:1024; attention is
    # permutation-invariant in j, so the rotated frame is safe.
    xb = nc.declare_dram_parameter("xb", [C, N], BF16, isOutput=False)
    cb = nc.declare_dram_parameter("cb", [C, N], BF16, isOutput=False)
    # packed weights [C, 2C]: Gt = (Wk^T Wq)^T | Mt = (Wp Wv)^T
    wall = nc.declare_dram_parameter("wall", [C, 2 * C], F32, isOutput=False)
    # packed per-channel vectors [C, 13]:
    # 0:g0 1:t0 2:g1 3:b1 4:g2 5:b2 6:ones 7-10:gavg 11:sel_e 12:sel_o
    vall = nc.declare_dram_parameter("vall", [C, 13], F32, isOutput=False)
    # bc4 group-broadcast indicator rows
    rall = nc.declare_dram_parameter("rall", [NG, C], F32, isOutput=False)
    # host-transposed x blocks: xtb[j, jb, c] = x[c, jb*128 + j]
    xtb = nc.declare_dram_parameter("xtb", [C, JB * C], BF16, isOutput=False)
    y = nc.declare_dram_parameter("y", [C, I], F32, isOutput=True)

    with SlimTC(nc) as tc:
        with (
            tc.tile_pool(name="persist", bufs=1) as per,
            tc.tile_pool(name="smalls", bufs=1) as sm,
            tc.tile_pool(name="ptiles", bufs=6) as pp,
        ):
            eps128_t = sm.tile([C, 1], F32, tag="eps128")
            nc.vector.memset(eps128_t[:], EPS)
            zero128_t = sm.tile([C, 1], F32, tag="zero128")
            nc.vector.memset(zero128_t[:], 0.0)
            zero1_t = sm.tile([1, 1], F32, tag="zero1")
            nc.vector.memset(zero1_t[:], 0.0)
            # warm the ACT table set (Ln+Exp) off the critical path
            warm_t = sm.tile([1, 1], F32, tag="warm")
            nc.vector.memset(warm_t[:], 1.0)
            nc.scalar.activation(out=warm_t[:], in_=warm_t[:],
                                 func=mybir.ActivationFunctionType.Ln,
                                 bias=zero1_t[:], scale=1.0)
            nc.scalar.activation(out=warm_t[:], in_=warm_t[:],
                                 func=mybir.ActivationFunctionType.Exp,
                                 bias=zero1_t[:], scale=1.0)

            # ---- packed constant loads (ahead of x/c in the sync queue) ----
            wall_t = per.tile([C, 2 * C], F32, tag="wall")
            nc.sync.dma_start(wall_t[:], wall[:])
            vall_t = sm.tile([C, 13], F32, tag="vall")
            nc.sync.dma_start(vall_t[:], vall[:])
            rall_t = sm.tile([NG, C], F32, tag="rall")
            nc.sync.dma_start(rall_t[:], rall[:])

            gt_t = wall_t[:, 0 * C:1 * C]
            mt_t = wall_t[:, 1 * C:2 * C]
            g0_t = vall_t[:, 0:1]
            t0_t = vall_t[:, 1:2]
            g1_t = vall_t[:, 2:3]
            b1_t = vall_t[:, 3:4]
            g2_t = vall_t[:, 4:5]
            b2_t = vall_t[:, 5:6]
            ones_t = vall_t[:, 6:7]
            gavg_t = vall_t[:, 7:11]
            bc4_t = rall_t[:]
            ones1_t = sm.tile([1, C], F32, tag="ones1")
            nc.vector.memset(ones1_t[:], 1.0)

            # rounded/typed copies for the PE
            gt_r = per.tile([C, C], MM_DT, tag="gt_r")
            nc.vector.tensor_copy(gt_r[:], gt_t[:])
            mt_r = per.tile([C, C], MM_DT, tag="mt_r")
            nc.vector.tensor_copy(mt_r[:], mt_t[:])
            ones_a = sm.tile([C, 1], ATTN_DT, tag="ones_a")
            nc.vector.tensor_copy(ones_a[:], ones_t[:])
            ones1_r = sm.tile([1, C], F32R, tag="ones1_r")
            nc.vector.tensor_copy(ones1_r[:], ones1_t[:])
            sele_r = sm.tile([C, 1], F32R, tag="sele_r")
            nc.vector.tensor_copy(sele_r[:], vall_t[:, 11:12])
            selo_r = sm.tile([C, 1], F32R, tag="selo_r")
            nc.vector.tensor_copy(selo_r[:], vall_t[:, 12:13])

            # ---- chunked input DMA + overlapped stats / transposes ----
            x_t = per.tile([C, N], BF16, tag="x")
            c_t = per.tile([C, N], BF16, tag="c")
            xt_t = per.tile([C, JB, C], BF16, tag="xt")
            stats_c = sm.tile([C, 8, 6], F32, tag="stats_c")
            scr_t = per.tile([C, I], F32, tag="scr")
            sxs = sm.tile([C, 2, 4], F32, tag="sxs")
            CH = 512

            # x/c interleaved halves first (stats gate everything), then the
            # host-transposed xt blocks -- all on the sync HW-DGE queue so xt
            # only uses DMA bandwidth after x/c have landed.  xt[jb] isn't
            # consumed until the AV matmul of iteration jb, ~2us in.
            for hf in range(2):
                hsl = slice(hf * (N // 2), (hf + 1) * (N // 2))
                nc.sync.dma_start(x_t[:, hsl], xb[:, hsl])
                nc.sync.dma_start(c_t[:, hsl], cb[:, hsl])
                for qr4 in range(2):
                    qr = hf * 2 + qr4
                    qsl = slice(qr * I, (qr + 1) * I)
                    # x sums on DVE, x sumsq on ScalarE -- parallel engines
                    nc.vector.tensor_reduce(
                        out=sxs[:, 0, qr:qr + 1], in_=x_t[:, qsl],
                        axis=mybir.AxisListType.X, op=mybir.AluOpType.add,
                    )
                    nc.scalar.activation(
                        out=scr_t[:], in_=x_t[:, qsl],
                        func=mybir.ActivationFunctionType.Square,
                        bias=zero128_t[:], scale=1.0,
                        accum_out=sxs[:, 1, qr:qr + 1],
                    )
                for ch4 in range(4):
                    ch = hf * 4 + ch4
                    nc.vector.bn_stats(
                        out=stats_c[:, ch, :],
                        in_=c_t[:, ch * CH:(ch + 1) * CH],
                    )
            for qt in range(4):
                qsl = slice(qt * 8, (qt + 1) * 8)
                nc.sync.dma_start(
                    xt_t[:, qsl, :], xtb[:, qt * 8 * C:(qt + 1) * 8 * C]
                )

            # ---- group-norm channel affine A[c], B[c] ----
            def gn_affine_d2(d2, gamma_t, beta_t, label):
                with tc.tile_pool(
                    name=f"gnps_{label}", bufs=1, space=bass.MemorySpace.PSUM
                ) as gnps:
                    gps = gnps.tile([NG, 2], F32, tag="g")
                    nc.tensor.matmul(gps[:], gavg_t[:], d2[:], start=True, stop=True)
                    gsb = sm.tile([NG, 2], F32, tag=f"gsb_{label}")
                    nc.vector.tensor_copy(gsb[:], gps[:])
                    cps = gnps.tile([C, 2], F32, tag="ch")
                    nc.tensor.matmul(cps[:], bc4_t[:], gsb[:], start=True, stop=True)
                    csb = sm.tile([C, 2], F32, tag=f"csb_{label}")
                    nc.vector.tensor_copy(csb[:], cps[:])
                var = sm.tile([C, 1], F32, tag=f"var_{label}")
                nc.vector.tensor_mul(var[:], csb[:, 0:1], csb[:, 0:1])
                nc.vector.tensor_sub(var[:], csb[:, 1:2], var[:])
                lnv = sm.tile([C, 1], F32, tag=f"lnv_{label}")
                nc.scalar.activation(
                    out=lnv[:], in_=var[:], func=mybir.ActivationFunctionType.Ln,
                    bias=eps128_t[:], scale=1.0,
                )
                rstd = sm.tile([C, 1], F32, tag=f"rstd_{label}")
                nc.scalar.activation(
                    out=rstd[:], in_=lnv[:], func=mybir.ActivationFunctionType.Exp,
                    bias=zero128_t[:], scale=-0.5,
                )
                a_t = sm.tile([C, 1], F32, tag=f"A_{label}")
                nc.vector.tensor_mul(a_t[:], rstd[:], gamma_t[:])
                b_t = sm.tile([C, 1], F32, tag=f"B_{label}")
                nc.vector.tensor_mul(b_t[:], csb[:, 0:1], a_t[:])
                nc.vector.tensor_sub(b_t[:], beta_t[:], b_t[:])
                return a_t, b_t

            # c path: bn_aggr -> [mean, mean^2+var]
            mv = sm.tile([C, 2], F32, tag="mv_c")
            nc.vector.bn_aggr(out=mv[:], in_=stats_c[:])
            d2c = sm.tile([C, 2], F32, tag="d2_c")
            nc.vector.tensor_copy(d2c[:, 0:1], mv[:, 0:1])
            nc.vector.tensor_mul(d2c[:, 1:2], mv[:, 0:1], mv[:, 0:1])
            nc.vector.tensor_add(d2c[:, 1:2], d2c[:, 1:2], mv[:, 1:2])
            ac_t, bc_t = gn_affine_d2(d2c, g2_t, b2_t, "c")

            # x path: chunk sums -> means
            d2x = sm.tile([C, 2], F32, tag="d2_x")
            nc.vector.tensor_reduce(
                out=d2x[:], in_=sxs[:],
                axis=mybir.AxisListType.X, op=mybir.AluOpType.add,
            )
            nc.vector.tensor_scalar_mul(d2x[:], d2x[:], 1.0 / float(N))
            ax_t, bx_t = gn_affine_d2(d2x, g1_t, b1_t, "x")

            # ---- q' = A1 * (G cn + g0) ----
            q_t = per.tile([C, I], ATTN_DT, tag="q")
            bf_t = sm.tile([C, 1], F32, tag="beta_f")
            s2_t = sm.tile([C, 1], F32, tag="s2")
            nc.vector.tensor_mul(s2_t[:], ax_t[:], g0_t[:])

            with tc.tile_pool(
                name="proj_ps", bufs=2, space=bass.MemorySpace.PSUM
            ) as pps:
                cn_t = per.tile([C, I], MM_DT, tag="cn")
                nc.vector.tensor_scalar(
                    out=cn_t[:], in0=c_t[:, 0:I], scalar1=ac_t[:], scalar2=bc_t[:],
                    op0=mybir.AluOpType.mult, op1=mybir.AluOpType.add,
                )
                qps = pps.tile([C, I], F32, tag="q")
                for ih in range(2):
                    nc.tensor.matmul(
                        qps[:, ih * 512:(ih + 1) * 512],
                        gt_r[:], cn_t[:, ih * 512:(ih + 1) * 512],
                        start=True, stop=True,
                    )
                nc.vector.tensor_scalar(
                    out=q_t[:], in0=qps[:], scalar1=ax_t[:], scalar2=s2_t[:],
                    op0=mybir.AluOpType.mult, op1=mybir.AluOpType.add,
                )
                # PE p-state warm fillers (no consumers; junk psum tile)
                jk_ps = pps.tile([C, 512], F32, tag="junk")
                for w in range(N_WARM):
                    nc.tensor.matmul(
                        jk_ps[:], x_t[:, 0:C], x_t[:, 0:512],
                        start=True, stop=True, skip_group_check=True,
                    )

            # ---- attention ----
            # Software-pipelined: scores for jb+2 are emitted ahead of the
            # exp-dependent AV/denom work for jb, so the PE never stalls on
            # the ScalarE exp.  Denominators accumulate into 4 separate
            # col-group accumulators (tile_position) so 4 ones-matmuls run
            # concurrently on the PE array.
            o_sb = per.tile([C, I], MM_DT, tag="osb")
            f_t = per.tile([C, I], F32, tag="f")
            zz_t = per.tile([C, I], F32, tag="zz")
            d4s = per.tile([C, 512], F32R, tag="d4s")

            st_tiles = {}
            p_tiles = {}

            with tc.tile_pool(
                name="acc_ps", bufs=1, space=bass.MemorySpace.PSUM
            ) as acc:
                o_ps = acc.tile([C, I], F32, tag="o")
                d4_ps = acc.tile([C, 512], F32, tag="d4")
                nc.vector.memset(d4_ps[:], 0.0)

                with tc.tile_pool(
                    name="st_ps", bufs=2, space=bass.MemorySpace.PSUM
                ) as stp:
                    def emit_scores(jb):
                        st = stp.tile([C, I], F32, tag="st")
                        st_tiles[jb] = st
                        for ih in range(2):
                            nc.tensor.matmul(
                                st[:, ih * 512:(ih + 1) * 512],
                                x_t[:, jb * 128:(jb + 1) * 128],
                                q_t[:, ih * 512:(ih + 1) * 512],
                                start=True, stop=True,
                            )

                    def emit_exp(jb):
                        p_t = pp.tile([C, I], ATTN_DT, tag="p")
                        p_tiles[jb] = p_t
                        nc.scalar.activation(
                            out=p_t[:], in_=st_tiles.pop(jb)[:],
                            func=mybir.ActivationFunctionType.Exp,
                            bias=zero128_t[:], scale=float(SCALE),
                        )

                    emit_scores(0)
                    # beta_f = M @ B1 + t0 (tail-only; emitted here so its
                    # A1/B1 wait never stalls the PE ahead of the warmups)
                    bf_ps = acc.tile([C, 1], F32, tag="bf")
                    nc.tensor.matmul(bf_ps[:], mt_t[:], bx_t[:],
                                     start=True, stop=True)
                    nc.vector.tensor_add(bf_t[:], bf_ps[:], t0_t[:])
                    emit_scores(1)
                    emit_exp(0)
                    for jb in range(JB):
                        if jb + 2 < JB:
                            emit_scores(jb + 2)
                        if jb + 1 < JB:
                            emit_exp(jb + 1)
                        p_t = p_tiles[jb]
                        first, last = jb == 0, jb == JB - 1
                        for ih in range(2):
                            sl = slice(ih * 512, (ih + 1) * 512)
                            nc.tensor.matmul(
                                o_ps[:, sl], xt_t[:, jb, :], p_t[:, sl],
                                start=first, stop=last,
                            )
                        if jb % 2 == 1:
                            for g in range(4):
                                jj, ih = jb - 1 + g // 2, g % 2
                                sl = slice(ih * 512, (ih + 1) * 512)
                                nc.tensor.matmul(
                                    d4_ps[32 * g:32 * g + 1, 0:512],
                                    ones_a[:], p_tiles[jj][:, sl],
                                    start=jb == 1, stop=last,
                                    tile_position=(0, 32 * g),
                                )
                            p_tiles.pop(jb - 1)
                            p_tiles.pop(jb)

                # U out of PSUM with the A1 row-scale folded in (DVE);
                # 2 halves so the z matmul starts after the first
                for ih in range(2):
                    sl = slice(ih * 512, (ih + 1) * 512)
                    nc.vector.tensor_scalar(
                        out=o_sb[:, sl], in0=o_ps[:, sl], scalar1=ax_t[:],
                        scalar2=zero128_t[:],
                        op0=mybir.AluOpType.mult, op1=mybir.AluOpType.add,
                    )

                # one copy moves all 4 denominator rows (other partitions
                # are memset zeros)
                nc.vector.tensor_copy(d4s[:], d4_ps[:])

            with tc.tile_pool(
                name="tail_ps", bufs=1, space=bass.MemorySpace.PSUM
            ) as tlp:
                d_fin = tlp.tile([1, I], F32, tag="dfin")
                # rows {0,64} hold i 0:512, rows {32,96} hold 512:1024
                nc.tensor.matmul(
                    d_fin[:, 0:512], sele_r[:], d4s[:], start=True, stop=True,
                )
                nc.tensor.matmul(
                    d_fin[:, 512:1024], selo_r[:], d4s[:], start=True, stop=True,
                )
                rsb = sm.tile([1, I], F32R, tag="rsb")
                with nc.allow_low_precision(reason="f32r is 32-bit"):
                    nc.vector.reciprocal(rsb[:], d_fin[:])

                z_ps = tlp.tile([C, I], F32, tag="z")
                rb_ps = tlp.tile([C, I], F32, tag="rb")
                rb_sb = per.tile([C, I], F32, tag="rbsb")
                for ih in range(2):
                    sl = slice(ih * 512, (ih + 1) * 512)
                    nc.tensor.matmul(z_ps[:, sl], mt_r[:], o_sb[:, sl],
                                     start=True, stop=True)
                    nc.tensor.matmul(
                        rb_ps[:, sl], ones1_r[:], rsb[:, sl],
                        start=True, stop=True,
                    )
                    # rb out of PSUM on the post-loop-idle ScalarE; combine
                    # on DVE: f = (z * recip + beta_f) + x
                    nc.scalar.activation(
                        out=rb_sb[:, sl], in_=rb_ps[:, sl],
                        func=mybir.ActivationFunctionType.Identity,
                        bias=zero128_t[:], scale=1.0,
                    )
                    nc.vector.tensor_tensor(
                        zz_t[:, sl], z_ps[:, sl], rb_sb[:, sl],
                        mybir.AluOpType.mult,
                    )
                    nc.vector.scalar_tensor_tensor(
                        out=f_t[:, sl], in0=zz_t[:, sl], scalar=bf_t[:],
                        in1=x_t[:, sl],
                        op0=mybir.AluOpType.add, op1=mybir.AluOpType.add,
                    )
                    nc.sync.dma_start(y[:, sl], f_t[:, sl])

    cap_sync_waits(nc)
    return nc


_PROGRAM = None


def _get_program():
    global _PROGRAM
    if _PROGRAM is None:
        _PROGRAM = build_program()
    return _PROGRAM


def _prep_in_maps(x, c, g1, b1, g2, b2, wq, bq, wk, bk, wv, bv, wp, bp):
    f = np.float32
    a = lambda v: np.asarray(v, f)
    ch = np.arange(C) // 32
    gavg = np.zeros((C, NG), f)
    gavg[np.arange(C), ch] = 1.0 / 32.0
    bc4 = np.zeros((NG, C), f)
    bc4[ch, np.arange(C)] = 1.0
    gt = a(wq).T @ a(wk)             # lhsT for q'' = (Wk^T Wq) @ cn
    mt = (a(wp) @ a(wv)).T           # lhsT for z = (Wp Wv) @ Us
    wall = np.concatenate([gt, mt], axis=1)
    g0 = a(wk).T @ a(bq)
    t0 = a(wp) @ a(bv) + a(bp)
    vall = np.stack([
        g0, t0, a(g1), a(b1), a(g2), a(b2), np.ones(C, f),
    ], axis=1)
    sel_e = np.zeros((C, 1), f); sel_e[0] = 1.0; sel_e[64] = 1.0
    sel_o = np.zeros((C, 1), f); sel_o[32] = 1.0; sel_o[96] = 1.0
    vall = np.concatenate([vall, gavg, sel_e, sel_o], axis=1)   # [C, 13]
    rall = bc4
    common = {
        "wall": np.ascontiguousarray(wall),
        "vall": np.ascontiguousarray(vall),
        "rall": np.ascontiguousarray(rall),
    }
    xf = a(x).reshape(2, C, N)
    cf = a(c).reshape(2, C, N)
    in_maps = []
    for m in range(N_CORES):
        b, quarter = m // 4, m % 4
        i0 = quarter * I
        # roll columns so this core's query/residual rows are columns 0:I;
        # attention is permutation-invariant in j so the rotated frame is safe
        xr = np.ascontiguousarray(
            np.roll(xf[b], -i0, axis=1)).astype(ml_dtypes.bfloat16)
        # xt[j, jb, c] = x[c, jb*128 + j]
        xt = np.ascontiguousarray(
            xr.reshape(C, JB, 128).transpose(2, 1, 0)).reshape(C, JB * C)
        in_maps.append({
            "xb": xr,
            "xtb": xt,
            "cb": np.ascontiguousarray(
                np.roll(cf[b], -i0, axis=1)).astype(ml_dtypes.bfloat16),
            **common,
        })
    return in_maps


def run_spmd(inputs, trace=False, **kw):
    nc = _get_program()
    in_maps = _prep_in_maps(**inputs)
    return run_bass_kernel_spmd(nc, in_maps, list(range(N_CORES)), trace=trace, **kw)


def kernel(**inputs) -> np.ndarray:
    res = run_spmd(inputs, trace=False)
    out = np.empty((2, C, N), np.float32)
    for m in range(N_CORES):
        b, quarter = m // 4, m % 4
        out[b][:, quarter * I:(quarter + 1) * I] = res.results[m]["y"]
    return out.reshape(2, C, 16, 16, 16)


# revision 23
# speedup vs baseline: 1.0769x; 1.0214x over previous
"""AttnBlock (GroupNorm + cross-attention + proj + residual) on 8 trn2 cores.

Problem (hardcoded shapes): x, c: [2, 128, 16, 16, 16] fp32; C=128 channels,
N=4096 spatial tokens, 4 groups of 32 channels.

  h  = GN(x; g1, b1)            c_ = GN(c; g2, b2)
  q = wq c_ + bq ; k = wk h + bk ; v = wv h + bv
  S[b,i,j] = <q[:,i], k[:,j]> / sqrt(C) ;  A = softmax_j(S)
  out = x + wp (v A^T) + bp

Folded formulation (x-side GroupNorm folded through the attention algebra,
so attention runs directly on RAW bf16 x):
  With D = diag(A1), hn = A1*x + B1 (per-channel affine from GN stats):
    scores  S'[j,i] = x[:,j]^T q'[:,i],  q' = D Wk^T q  (the per-i constant
            beta_k^T q drops out of softmax_j)
    q = Wq cn + bq  =>  q' = D (G cn + g0),  G = Wk^T Wq, g0 = Wk^T bq
    U = X P  (raw x as values);  out = x + M D (U/d) + beta_f,
            M = Wp Wv,  beta_f = M B1 + Wp bv + bp
  Host packs G^T, M^T and the group-mean projector Gproj = gavg @ bc4.

Sharding: 8 cores, core m -> batch b=m//4, query rows i0=(m%4)*1024 .. +1024.
Inputs arrive column-rotated so the core's query/residual slice is cols 0:I.
Per core: DMA raw bf16 x (+ host-transposed xt blocks) on the sync HW-DGE
stream, c on the gpsimd stream in parallel; GN stats overlap the DMA; then a
software-pipelined scores->exp->AV loop (exp on ACT is the wall; PE warmup
matmuls gated mid-affine keep the p-state ramp off the loop), then a short
tail: denominators live in 2 PSUM rows {0,64} (2-group tile_position), ACT
Ln/Exp reciprocal, K=2 broadcast matmuls, DVE combine, DMA out.
"""

import ml_dtypes
import numpy as np

import concourse.bass as bass
import concourse.tile as tile
from concourse import mybir
from concourse.bass_utils import run_bass_kernel_spmd

N_CORES = 8
C = 128
N = 4096          # tokens per batch
I = 1024          # query rows per core
NG = 4            # groups
EPS = 1e-6
SCALE = 1.0 / np.sqrt(C)
JB = N // 128     # 32 j-blocks
F32 = mybir.dt.float32
F32R = mybir.dt.float32r
BF16 = mybir.dt.bfloat16

MM_DT = BF16
ATTN_DT = BF16

N_WARM = 3        # PE p-state warm matmuls gated on the mid-affine wsrc copy

DEBUG_OUTS = False


class SlimTC(tile.TileContext):
    """TileContext with a slimmer kernel-tail: one all-engine barrier instead
    of two.  The second barrier only orders other engines behind the sem
    clears; each engine's own stream still completes before halt, and NRT
    doesn't restart streams until all engines halt, so reruns stay safe."""

    def _drain_and_barrier(self, tick_clock, wait_clock):
        from concourse.vector_clock import ScopedClock
        drain_inst = self.nc.sync.drain()
        wait_clock.add_sem_waits(
            drain_inst.ins, ScopedClock({None: tick_clock.global_clock})
        )
        done = self.nc.alloc_semaphore("tail_done")
        drain_inst.then_inc(done)
        self.nc.gpsimd.wait_ge(done, 1)
        assert self.sems is not None
        popped = self.nc._tile_sem_poison_stack.pop()
        assert popped is self._sem_poison
        sems = list(self.sems.allocated().values()) + [done]
        self.nc.clear_and_free_semaphores(sems)


def cap_sync_waits(nc):
    """Split multi-wait instructions: the pinned walrus accepts at most one
    sync wait per instruction ("Too many sync wait commands"). Hoist extra
    waits into single-wait NOPs inserted just before, on the same engine."""
    ctr = 0
    for f in nc.m.functions:
        for b in f.blocks:
            out = []
            for inst in b.instructions:
                si = inst.sync_info
                if si is not None and si.on_wait and len(si.on_wait) > 1:
                    waits = list(si.on_wait)
                    for w in waits[:-1]:
                        ctr += 1
                        out.append(mybir.InstNoOp(
                            name=f"I-waitsplit-{ctr}",
                            engine=inst.engine,
                            bass_nofuse=True,
                            sync_info=mybir.SyncInfo(on_wait=[w], on_update=[]),
                        ))
                    si.on_wait = waits[-1:]
                out.append(inst)
            b.instructions = out


def build_program():
    nc = bass.Bass("TRN2", target_bir_lowering=False, debug=False)

    # I/O.  xb/cb arrive ROTATED per core (columns rolled by -i0) so the
    # query/residual slice is always columns 0:1024; attention is
    # permutation-invariant in j, so the rotated frame is safe.
    xb = nc.declare_dram_parameter("xb", [C, N], BF16, isOutput=False)
    cb = nc.declare_dram_parameter("cb", [C, N], BF16, isOutput=False)
    # packed weights [C, 3C]: Gt = (Wk^T Wq)^T | Mt = (Wp Wv)^T | Gproj
    wall = nc.declare_dram_parameter("wall", [C, 3 * C], F32, isOutput=False)
    # packed per-channel vectors [C, 7]: 0:g0 1:t0 2:g1 3:b1 4:g2 5:b2 6:ones
    vall = nc.declare_dram_parameter("vall", [C, 7], F32, isOutput=False)
    # host-transposed x blocks: xtb[j, jb*C + c] = x[c, jb*128 + j]
    xtb = nc.declare_dram_parameter("xtb", [C, JB * C], BF16, isOutput=False)
    y = nc.declare_dram_parameter("y", [C, I], F32, isOutput=True)

    with SlimTC(nc) as tc:
        with (
            tc.tile_pool(name="persist", bufs=1) as per,
            tc.tile_pool(name="smalls", bufs=1) as sm,
            tc.tile_pool(name="ptiles", bufs=6) as pp,
        ):
            eps128_t = sm.tile([C, 1], F32, tag="eps128")
            nc.vector.memset(eps128_t[:], EPS)
            zero128_t = sm.tile([C, 1], F32, tag="zero128")
            nc.vector.memset(zero128_t[:], 0.0)
            zero1_t = sm.tile([2, 1], F32, tag="zero1")
            nc.vector.memset(zero1_t[:], 0.0)
            # warm the ACT table set (Ln+Exp) off the critical path
            warm_t = sm.tile([1, 1], F32, tag="warm")
            nc.vector.memset(warm_t[:], 1.0)
            nc.scalar.activation(out=warm_t[:], in_=warm_t[:],
                                 func=mybir.ActivationFunctionType.Ln,
                                 bias=zero1_t[0:1, :], scale=1.0)
            nc.scalar.activation(out=warm_t[:], in_=warm_t[:],
                                 func=mybir.ActivationFunctionType.Exp,
                                 bias=zero1_t[0:1, :], scale=1.0)
            # K=1 all-ones lhsT for the reciprocal broadcast matmuls
            ones1_t = sm.tile([1, C], F32, tag="ones1")
            nc.vector.memset(ones1_t[:], 1.0)
            ones1_r = sm.tile([1, C], F32R, tag="ones1_r")
            nc.vector.tensor_copy(ones1_r[:], ones1_t[:])

            # ---- packed constant loads (ahead of x/xt in the sync queue) ----
            wall_t = per.tile([C, 3 * C], F32, tag="wall")
            nc.sync.dma_start(wall_t[:], wall[:])
            vall_t = sm.tile([C, 7], F32, tag="vall")
            nc.sync.dma_start(vall_t[:], vall[:])

            gt_t = wall_t[:, 0 * C:1 * C]
            mt_t = wall_t[:, 1 * C:2 * C]
            gproj_t = wall_t[:, 2 * C:3 * C]
            g0_t = vall_t[:, 0:1]
            t0_t = vall_t[:, 1:2]
            g1_t = vall_t[:, 2:3]
            b1_t = vall_t[:, 3:4]
            g2_t = vall_t[:, 4:5]
            b2_t = vall_t[:, 5:6]
            ones_t = vall_t[:, 6:7]

            # rounded/typed copies for the PE
            gt_r = per.tile([C, C], MM_DT, tag="gt_r")
            nc.vector.tensor_copy(gt_r[:], gt_t[:])
            mt_r = per.tile([C, C], MM_DT, tag="mt_r")
            nc.vector.tensor_copy(mt_r[:], mt_t[:])
            ones_a = sm.tile([C, 1], ATTN_DT, tag="ones_a")
            nc.vector.tensor_copy(ones_a[:], ones_t[:])

            # ---- input DMA + overlapped stats ----
            # x halves then xt quarters on the sync HW-DGE stream; c halves
            # on the gpsimd SW-DGE stream in parallel.  xt only uses DMA
            # bandwidth after x has landed; xt[jb] isn't consumed until the
            # AV matmul of iteration jb.
            x_t = per.tile([C, N], BF16, tag="x")
            c_t = per.tile([C, N], BF16, tag="c")
            xt_t = per.tile([C, JB, C], BF16, tag="xt")
            stats_c = sm.tile([C, 8, 6], F32, tag="stats_c")
            scr_t = per.tile([C, I], F32, tag="scr")
            sxs = sm.tile([C, 2, 4], F32, tag="sxs")
            CH = 512

            for hf in range(2):
                hsl = slice(hf * (N // 2), (hf + 1) * (N // 2))
                nc.sync.dma_start(x_t[:, hsl], xb[:, hsl])
                nc.gpsimd.dma_start(c_t[:, hsl], cb[:, hsl])
                for qr4 in range(2):
                    qr = hf * 2 + qr4
                    qsl = slice(qr * I, (qr + 1) * I)
                    # x sums on DVE, x sumsq on ScalarE -- parallel engines
                    nc.vector.tensor_reduce(
                        out=sxs[:, 0, qr:qr + 1], in_=x_t[:, qsl],
                        axis=mybir.AxisListType.X, op=mybir.AluOpType.add,
                    )
                    nc.scalar.activation(
                        out=scr_t[:], in_=x_t[:, qsl],
                        func=mybir.ActivationFunctionType.Square,
                        bias=zero128_t[:], scale=1.0,
                        accum_out=sxs[:, 1, qr:qr + 1],
                    )
            for qt in range(4):
                qsl = slice(qt * 8, (qt + 1) * 8)
                nc.sync.dma_start(
                    xt_t[:, qsl, :], xtb[:, qt * 8 * C:(qt + 1) * 8 * C]
                )
            for ch in range(8):
                nc.vector.bn_stats(
                    out=stats_c[:, ch, :],
                    in_=c_t[:, ch * CH:(ch + 1) * CH],
                )

            # ---- group-norm channel affine A[c], B[c] ----
            # d2 = [mean_c, E[x^2]_c]; one Gproj matmul aggregates+broadcasts
            # group means; var ops read the PSUM result directly;
            # rstd = exp(-0.5*ln(var+eps)).
            def gn_affine_d2(gnps, d2, gamma_t, beta_t, label):
                cps = gnps.tile([C, 2], F32, tag=f"ch_{label}")
                nc.tensor.matmul(cps[:], gproj_t[:], d2[:], start=True, stop=True)
                csb = sm.tile([C, 2], F32, tag=f"csb_{label}")
                nc.vector.tensor_copy(csb[:], cps[:])
                var = sm.tile([C, 1], F32, tag=f"var_{label}")
                nc.vector.tensor_mul(var[:], csb[:, 0:1], csb[:, 0:1])
                nc.vector.tensor_sub(var[:], csb[:, 1:2], var[:])
                return csb, var

            def gn_affine_fin(cps, var, gamma_t, beta_t, label):
                lnv = sm.tile([C, 1], F32, tag=f"lnv_{label}")
                nc.scalar.activation(
                    out=lnv[:], in_=var[:], func=mybir.ActivationFunctionType.Ln,
                    bias=eps128_t[:], scale=1.0,
                )
                rstd = sm.tile([C, 1], F32, tag=f"rstd_{label}")
                nc.scalar.activation(
                    out=rstd[:], in_=lnv[:], func=mybir.ActivationFunctionType.Exp,
                    bias=zero128_t[:], scale=-0.5,
                )
                a_t = sm.tile([C, 1], F32, tag=f"A_{label}")
                nc.vector.tensor_mul(a_t[:], rstd[:], gamma_t[:])
                b_t = sm.tile([C, 1], F32, tag=f"B_{label}")
                nc.vector.tensor_mul(b_t[:], cps[:, 0:1], a_t[:])
                nc.vector.tensor_sub(b_t[:], beta_t[:], b_t[:])
                return a_t, b_t

            q_t = per.tile([C, I], ATTN_DT, tag="q")
            bf_t = sm.tile([C, 1], F32, tag="beta_f")
            s2_t = sm.tile([C, 1], F32, tag="s2")
            wsrc = sm.tile([C, 512], BF16, tag="wsrc")

            with tc.tile_pool(
                name="gn_ps", bufs=1, space=bass.MemorySpace.PSUM
            ) as gnps:
                # x path: chunk sums -> means (DVE), then Gproj matmul
                d2x = sm.tile([C, 2], F32, tag="d2_x")
                nc.vector.tensor_reduce(
                    out=d2x[:], in_=sxs[:],
                    axis=mybir.AxisListType.X, op=mybir.AluOpType.add,
                )
                nc.vector.tensor_scalar_mul(d2x[:], d2x[:], 1.0 / float(N))
                cpx, varx = gn_affine_d2(gnps, d2x, g1_t, b1_t, "x")
                # warm-source copy: gates the PE warm matmuls so they run
                # during the Ln/Exp rstd window, right before the loop
                nc.vector.tensor_copy(wsrc[:], x_t[:, 0:512])
                ax_t, bx_t = gn_affine_fin(cpx, varx, g1_t, b1_t, "x")
                nc.vector.tensor_mul(s2_t[:], ax_t[:], g0_t[:])

                # c path: bn_aggr -> [mean, mean^2+var]
                mv = sm.tile([C, 2], F32, tag="mv_c")
                nc.vector.bn_aggr(out=mv[:], in_=stats_c[:])
                d2c = sm.tile([C, 2], F32, tag="d2_c")
                nc.vector.tensor_copy(d2c[:, 0:1], mv[:, 0:1])
                nc.vector.tensor_mul(d2c[:, 1:2], mv[:, 0:1], mv[:, 0:1])
                nc.vector.tensor_add(d2c[:, 1:2], d2c[:, 1:2], mv[:, 1:2])
                cpc, varc = gn_affine_d2(gnps, d2c, g2_t, b2_t, "c")
                ac_t, bc_t = gn_affine_fin(cpc, varc, g2_t, b2_t, "c")

                # ---- q' = A1 * (G cn + g0), in 512-halves ----
                cn_t = per.tile([C, I], MM_DT, tag="cn")
                for ih in range(2):
                    sl = slice(ih * 512, (ih + 1) * 512)
                    nc.vector.tensor_scalar(
                        out=cn_t[:, sl], in0=c_t[:, sl], scalar1=ac_t[:],
                        scalar2=bc_t[:],
                        op0=mybir.AluOpType.mult, op1=mybir.AluOpType.add,
                    )

            with tc.tile_pool(
                name="proj_ps", bufs=2, space=bass.MemorySpace.PSUM
            ) as pps:
                # PE p-state warm fillers (no consumers; junk psum tile)
                jk_ps = pps.tile([C, 512], F32, tag="junk")
                for w in range(N_WARM):
                    nc.tensor.matmul(
                        jk_ps[:], x_t[:, 0:C], wsrc[:],
                        start=True, stop=True, skip_group_check=True,
                    )
                qps = pps.tile([C, I], F32, tag="q")
                for ih in range(2):
                    sl = slice(ih * 512, (ih + 1) * 512)
                    nc.tensor.matmul(
                        qps[:, sl], gt_r[:], cn_t[:, sl],
                        start=True, stop=True,
                    )
                    nc.vector.tensor_scalar(
                        out=q_t[:, sl], in0=qps[:, sl], scalar1=ax_t[:],
                        scalar2=s2_t[:],
                        op0=mybir.AluOpType.mult, op1=mybir.AluOpType.add,
                    )

            # ---- attention ----
            # Software-pipelined: scores for jb+2 are emitted ahead of the
            # exp-dependent AV/denom work for jb, so the PE never stalls on
            # the ScalarE exp.  Denominators accumulate into 2 col-group
            # accumulators (tile_position) landing in PSUM rows {0, 64}:
            # row 0 = d[0:512], row 64 = d[512:1024] -- no pair-sum needed.
            o_sb = per.tile([C, I], MM_DT, tag="osb")
            f_t = per.tile([C, I], F32, tag="f")
            zz_t = per.tile([C, I], F32, tag="zz")
            rsb = [sm.tile([1, 512], F32R, tag=f"rsb{ih}", name=f"rsb{ih}")
                   for ih in range(2)]
            rb_sb = per.tile([C, I], F32, tag="rbsb")

            st_tiles = {}
            p_tiles = {}

            with tc.tile_pool(
                name="acc_ps", bufs=1, space=bass.MemorySpace.PSUM
            ) as acc:
                o_ps = acc.tile([C, I], F32, tag="o")
                d4_ps = acc.tile([C, 512], F32, tag="d4")
                nc.vector.memset(d4_ps[:], 0.0)

                with tc.tile_pool(
                    name="st_ps", bufs=2, space=bass.MemorySpace.PSUM
                ) as stp:
                    def emit_scores(jb):
                        st = stp.tile([C, I], F32, tag="st")
                        st_tiles[jb] = st
                        for ih in range(2):
                            nc.tensor.matmul(
                                st[:, ih * 512:(ih + 1) * 512],
                                x_t[:, jb * 128:(jb + 1) * 128],
                                q_t[:, ih * 512:(ih + 1) * 512],
                                start=True, stop=True,
                            )

                    def emit_exp(jb):
                        p_t = pp.tile([C, I], ATTN_DT, tag="p")
                        p_tiles[jb] = p_t
                        nc.scalar.activation(
                            out=p_t[:], in_=st_tiles.pop(jb)[:],
                            func=mybir.ActivationFunctionType.Exp,
                            bias=zero128_t[:], scale=float(SCALE),
                        )

                    emit_scores(0)
                    # beta_f = M @ B1 + t0 (tail-only; emitted here so its
                    # B1 wait never stalls the PE ahead of the warmups)
                    bf_ps = acc.tile([C, 1], F32, tag="bf")
                    nc.tensor.matmul(bf_ps[:], mt_t[:], bx_t[:],
                                     start=True, stop=True)
                    nc.vector.tensor_add(bf_t[:], bf_ps[:], t0_t[:])
                    emit_scores(1)
                    emit_exp(0)
                    for jb in range(JB):
                        if jb + 2 < JB:
                            emit_scores(jb + 2)
                        if jb + 1 < JB:
                            emit_exp(jb + 1)
                        p_t = p_tiles[jb]
                        first, last = jb == 0, jb == JB - 1
                        for ih in range(2):
                            sl = slice(ih * 512, (ih + 1) * 512)
                            nc.tensor.matmul(
                                o_ps[:, sl], xt_t[:, jb, :], p_t[:, sl],
                                start=first, stop=last,
                            )
                        if jb % 2 == 1:
                            for ih in range(2):
                                sl = slice(ih * 512, (ih + 1) * 512)
                                for t, jj in enumerate((jb - 1, jb)):
                                    nc.tensor.matmul(
                                        d4_ps[64 * ih:64 * ih + 1, 0:512],
                                        ones_a[:], p_tiles[jj][:, sl],
                                        start=jb == 1 and t == 0,
                                        stop=last and t == 1,
                                        tile_position=(0, 64 * ih),
                                    )
                            p_tiles.pop(jb - 1)
                            p_tiles.pop(jb)

                # U out of PSUM with the A1 row-scale folded in (DVE);
                # 2 halves so the z matmul starts after the first
                for ih in range(2):
                    sl = slice(ih * 512, (ih + 1) * 512)
                    nc.vector.tensor_scalar(
                        out=o_sb[:, sl], in0=o_ps[:, sl], scalar1=ax_t[:],
                        scalar2=zero128_t[:],
                        op0=mybir.AluOpType.mult, op1=mybir.AluOpType.add,
                    )

                # reciprocal 1/d on ACT (idle post-loop): Ln then Exp(-1),
                # per denominator row ({0} = d[0:512], {64} = d[512:1024])
                # so the ih=0 tail chain starts after only one Ln+Exp
                for ih in range(2):
                    row = 64 * ih
                    lnd = sm.tile([1, 512], F32, tag=f"lnd{ih}")
                    nc.scalar.activation(
                        out=lnd[:], in_=d4_ps[row:row + 1, 0:512],
                        func=mybir.ActivationFunctionType.Ln,
                        bias=zero1_t[0:1, :], scale=1.0,
                    )
                    nc.scalar.activation(
                        out=rsb[ih][:], in_=lnd[:],
                        func=mybir.ActivationFunctionType.Exp,
                        bias=zero1_t[0:1, :], scale=-1.0,
                    )

            with tc.tile_pool(
                name="tail_ps", bufs=1, space=bass.MemorySpace.PSUM
            ) as tlp:
                z_ps = tlp.tile([C, I], F32, tag="z")
                rb_ps = tlp.tile([C, I], F32, tag="rb")
                for ih in range(2):
                    sl = slice(ih * 512, (ih + 1) * 512)
                    nc.tensor.matmul(z_ps[:, sl], mt_r[:], o_sb[:, sl],
                                     start=True, stop=True)
                for ih in range(2):
                    sl = slice(ih * 512, (ih + 1) * 512)
                    nc.tensor.matmul(
                        rb_ps[:, sl], ones1_r[:], rsb[ih][:],
                        start=True, stop=True,
                    )
                    # rb out of PSUM on the post-loop-idle ScalarE; combine
                    # on DVE: f = (z * recip + beta_f) + x
                    nc.scalar.activation(
                        out=rb_sb[:, sl], in_=rb_ps[:, sl],
                        func=mybir.ActivationFunctionType.Identity,
                        bias=zero128_t[:], scale=1.0,
                    )
                    nc.vector.tensor_tensor(
                        zz_t[:, sl], z_ps[:, sl], rb_sb[:, sl],
                        mybir.AluOpType.mult,
                    )
                    nc.vector.scalar_tensor_tensor(
                        out=f_t[:, sl], in0=zz_t[:, sl], scalar=bf_t[:],
                        in1=x_t[:, sl],
                        op0=mybir.AluOpType.add, op1=mybir.AluOpType.add,
                    )
                    nc.sync.dma_start(y[:, sl], f_t[:, sl])

    cap_sync_waits(nc)
    return nc


_PROGRAM = None


def _get_program():
    global _PROGRAM
    if _PROGRAM is None:
        _PROGRAM = build_program()
    return _PROGRAM


def _prep_in_maps(x, c, g1, b1, g2, b2, wq, bq, wk, bk, wv, bv, wp, bp):
    f = np.float32
    a = lambda v: np.asarray(v, f)
    ch = np.arange(C) // 32
    gproj = (ch[:, None] == ch[None, :]).astype(f) / 32.0
    gt = a(wq).T @ a(wk)             # lhsT for q'' = (Wk^T Wq) @ cn
    mt = (a(wp) @ a(wv)).T           # lhsT for z = (Wp Wv) @ Us
    wall = np.concatenate([gt, mt, gproj], axis=1)
    g0 = a(wk).T @ a(bq)
    t0 = a(wp) @ a(bv) + a(bp)
    vall = np.stack([
        g0, t0, a(g1), a(b1), a(g2), a(b2), np.ones(C, f),
    ], axis=1)                       # [C, 7]
    common = {
        "wall": np.ascontiguousarray(wall),
        "vall": np.ascontiguousarray(vall),
    }
    xf = a(x).reshape(2, C, N)
    cf = a(c).reshape(2, C, N)
    in_maps = []
    for m in range(N_CORES):
        b, quarter = m // 4, m % 4
        i0 = quarter * I
        # roll columns so this core's query/residual rows are columns 0:I;
        # attention is permutation-invariant in j so the rotated frame is safe
        xr = np.ascontiguousarray(
            np.roll(xf[b], -i0, axis=1)).astype(ml_dtypes.bfloat16)
        # xt[j, jb, c] = x[c, jb*128 + j]
        xt = np.ascontiguousarray(
            xr.reshape(C, JB, 128).transpose(2, 1, 0)).reshape(C, JB * C)
        in_maps.append({
            "xb": xr,
            "xtb": xt,
            "cb": np.ascontiguousarray(
                np.roll(cf[b], -i0, axis=1)).astype(ml_dtypes.bfloat16),
            **common,
        })
    return in_maps


def run_spmd(inputs, trace=False, **kw):
    nc = _get_program()
    in_maps = _prep_in_maps(**inputs)
    return run_bass_kernel_spmd(nc, in_maps, list(range(N_CORES)), trace=trace, **kw)


def kernel(**inputs) -> np.ndarray:
    res = run_spmd(inputs, trace=False)
    out = np.empty((2, C, N), np.float32)
    for m in range(N_CORES):
        b, quarter = m // 4, m % 4
        out[b][:, quarter * I:(quarter + 1) * I] = res.results[m]["y"]
    return out.reshape(2, C, 16, 16, 16)
